# revision 7
# baseline (speedup 1.0000x reference)
"""MoE layer (E=8 experts, top-2) on 8 Trainium2 NeuronCores.

Strategy: expert parallelism with host-side routing (the host plays the role
of the all-to-all token dispatch in the sharding hint, exactly like the
host-side combine-sum). Core c holds expert c's weights. The host computes
the router (16 MFLOP), gathers each expert's routed tokens into a fixed
capacity-C buffer, and each core runs the dense FFN over its C tokens:

    yT_c = (gelu(xG_c @ w1_c + b1_c) @ w2_c + b2_c).T     [H, C]

The host scatters the per-core outputs back to token order, scaled by the
top-2 softmax combine weights, and sums the two expert contributions.

Numerics: weights and activations in fp16 (halves weight DMA traffic, which
is otherwise the bottleneck at ~360 GB/s), all matmul accumulation in fp32
PSUM, bias + gelu in fp32 on the scalar engine. Output written in fp32.
"""

import numpy as np

import concourse.mybir as mybir
from concourse import bacc
from concourse.bass_utils import run_bass_kernel_spmd
from concourse.tile import TileContext

FP32 = mybir.dt.float32
FP16 = mybir.dt.float16
AF = mybir.ActivationFunctionType

P = 128
T, H, F, E = 1024, 1024, 4096, 8
HT, FT = H // P, F // P
N_CORES = 8

C_DEFAULT = 272   # expert capacity (max routed load for the fixed input)
NWARM = 5         # PE warmup matmuls to ramp the clock while DMAs land

_cache = {}


def _build_v5(C, act_fn=None):
    act_fn = AF.Gelu if act_fn is None else act_fn
    nc = bacc.Bacc()

    xg = nc.declare_dram_parameter("xg", [P, HT * C], FP16, isOutput=False)
    w1p = nc.declare_dram_parameter("w1p", [P, FT * HT * P], FP16, isOutput=False)
    w2p = nc.declare_dram_parameter("w2p", [P, HT * FT * P], FP16, isOutput=False)
    b1t = nc.declare_dram_parameter("b1t", [P, FT], FP32, isOutput=False)
    b2t = nc.declare_dram_parameter("b2t", [P, HT], FP32, isOutput=False)
    outp = nc.declare_dram_parameter("outp", [H, C], FP32, isOutput=True)

    xg3 = xg.rearrange("p (ht c) -> p ht c", ht=HT)
    w1_4d = w1p.rearrange("p (ft ht fl) -> p ft ht fl", ft=FT, ht=HT)
    w2_4d = w2p.rearrange("p (hh ft hl) -> p hh ft hl", hh=HT, ft=FT)

    # first chunk small so phase-A compute can start early; uniform 2-ft
    # chunks keep DMA supply (~0.73us/ft) ahead of PE demand (~0.91us/ft)
    # with fine availability granularity (each chunk +900ns sem lag)
    w1_chunks = [(0, 1)] + [(a, min(a + 2, FT)) for a in range(1, FT, 2)]

    with TileContext(nc) as tc:
        with (
            tc.tile_pool(name="const", bufs=1) as const,
            tc.tile_pool(name="wpool", bufs=1) as wpool,
            tc.tile_pool(name="hpool", bufs=1) as hpool,
            tc.tile_pool(name="opool", bufs=3) as opool,
            tc.tile_pool(name="psA", bufs=2, space="PSUM") as psA,
            tc.tile_pool(name="psB", bufs=2, space="PSUM") as psB,
            tc.tile_pool(name="psW", bufs=1, space="PSUM") as psW,
        ):
            # PE warmup: dummy matmuls keep the tensor engine busy (and its
            # p-state ramping) while the first weight/activation DMAs land.
            wmv = const.tile([P, 512], FP16)
            nc.vector.memset(wmv, 0.0)
            pw = psW.tile([P, 512], FP32)
            for _ in range(NWARM):
                nc.tensor.matmul(pw, wmv[:, :P], wmv, start=True, stop=True)

            # DMA issue order == transfer order: first w1 tile + x first so
            # phase-A compute starts as early as possible.
            w1sb = wpool.tile([P, FT, HT, P], FP16)
            a, b = w1_chunks[0]
            nc.sync.dma_start(out=w1sb[:, a:b, :, :], in_=w1_4d[:, a:b, :, :])
            xsb = const.tile([P, HT, C], FP16)
            nc.sync.dma_start(out=xsb[:, : HT // 2, :], in_=xg3[:, : HT // 2, :])
            nc.sync.dma_start(out=xsb[:, HT // 2 :, :], in_=xg3[:, HT // 2 :, :])
            b1_sb = const.tile([P, FT], FP32)
            b2_sb = const.tile([P, HT], FP32)
            for i, (a, b) in enumerate(w1_chunks[1:]):
                nc.sync.dma_start(out=w1sb[:, a:b, :, :], in_=w1_4d[:, a:b, :, :])
                if i == 1:
                    # biases are tiny; slot them in behind the first chunks
                    # (needed only by the first activation, ~7us in)
                    nc.sync.dma_start(out=b1_sb, in_=b1t[:, :])
                    nc.sync.dma_start(out=b2_sb, in_=b2t[:, :])
            w2sb = wpool.tile([P, HT, FT, P], FP16)
            for hh in range(HT):
                nc.sync.dma_start(out=w2sb[:, hh, :, :], in_=w2_4d[:, hh, :, :])

            hG = hpool.tile([P, FT, C], FP16)

            # phase A: hG[f, c] = gelu(w1.T @ xG.T + b1), fp16 out
            for ft in range(FT):
                pa = psA.tile([P, C], FP32, tag="pa", name="pa")
                for ht in range(HT):
                    nc.tensor.matmul(
                        pa,
                        w1sb[:, ft, ht, :],
                        xsb[:, ht, :],
                        start=(ht == 0),
                        stop=(ht == HT - 1),
                    )
                nc.scalar.activation(
                    hG[:, ft, :], pa, act_fn, bias=b1_sb[:, ft : ft + 1]
                )

            # phase B: yT[h', c] = w2.T @ hG + b2, fp32 out to DRAM
            for hh in range(HT):
                pb = psB.tile([P, C], FP32, tag="pb", name="pb")
                for ft in range(FT):
                    nc.tensor.matmul(
                        pb,
                        w2sb[:, hh, ft, :],
                        hG[:, ft, :],
                        start=(ft == 0),
                        stop=(ft == FT - 1),
                    )
                yt = opool.tile([P, C], FP32, tag="yt", name="yt")
                nc.scalar.activation(
                    yt, pb, AF.Identity, bias=b2_sb[:, hh : hh + 1]
                )
                nc.sync.dma_start(out=outp[P * hh : P * (hh + 1), :], in_=yt)

    nc.compile()
    return nc


def _get_nc(C=C_DEFAULT):
    key = ("v5", C)
    if key not in _cache:
        _cache[key] = _build_v5(C)
    return _cache[key]


def _route(x_flat, gate_w, gate_b):
    """Top-2 routing on host. Returns per-expert (token idx, combine wt)."""
    logits = x_flat @ gate_w.T + gate_b  # (T, E) fp32
    sel = np.argsort(-logits, axis=1, kind="stable")[:, :2]  # (T, 2)
    tw = np.take_along_axis(logits, sel, axis=1)
    tw = tw - tw.max(axis=1, keepdims=True)
    ew = np.exp(tw)
    rw = ew / ew.sum(axis=1, keepdims=True)  # (T, 2)
    idxs, wts = [], []
    for e in range(E):
        m = sel == e  # (T, 2)
        tok = np.nonzero(m.any(axis=1))[0]
        wt = rw[m.any(axis=1), :][m[m.any(axis=1), :]]
        idxs.append(tok)
        wts.append(wt.astype(np.float32))
    return idxs, wts


def kernel(x, gate_w, gate_b, w1, b1, w2, b2):
    x = np.asarray(x, dtype=np.float32)
    gate_w = np.asarray(gate_w, dtype=np.float32)
    gate_b = np.asarray(gate_b, dtype=np.float32)
    w1 = np.asarray(w1, dtype=np.float32)
    b1 = np.asarray(b1, dtype=np.float32)
    w2 = np.asarray(w2, dtype=np.float32)
    b2 = np.asarray(b2, dtype=np.float32)

    x_flat = x.reshape(T, H)
    idxs, wts = _route(x_flat, gate_w, gate_b)

    max_load = max(len(i) for i in idxs)
    C = C_DEFAULT if max_load <= C_DEFAULT else (max_load + 31) // 32 * 32
    nc = _get_nc(C)

    maps = []
    for c in range(N_CORES):
        tok = idxs[c]
        xg = np.zeros((C, H), dtype=np.float16)
        xg[: len(tok)] = x_flat[tok]
        # [C, H] -> [p, ht, c]
        xgp = np.ascontiguousarray(
            xg.reshape(C, HT, P).transpose(2, 1, 0)
        ).reshape(P, HT * C)
        w1c = w1[c].astype(np.float16)  # [H, F]
        w1pk = np.ascontiguousarray(
            w1c.reshape(HT, P, FT, P).transpose(1, 2, 0, 3)
        ).reshape(P, FT * HT * P)
        w2c = w2[c].astype(np.float16)  # [F, H]
        w2pk = np.ascontiguousarray(
            w2c.reshape(FT, P, HT, P).transpose(1, 2, 0, 3)
        ).reshape(P, HT * FT * P)
        maps.append(
            {
                "xg": xgp,
                "w1p": w1pk,
                "w2p": w2pk,
                "b1t": np.ascontiguousarray(b1[c].reshape(FT, P).T),
                "b2t": np.ascontiguousarray(b2[c].reshape(HT, P).T),
            }
        )

    res = run_bass_kernel_spmd(nc, maps, list(range(N_CORES)))

    out = np.zeros((T, H), dtype=np.float64)
    for c in range(N_CORES):
        yT = res.results[c]["outp"]  # [H, C] fp32
        n = len(idxs[c])
        out[idxs[c]] += wts[c][:, None].astype(np.float64) * yT[:, :n].T
    return out.astype(np.float32).reshape(1, T, H)


# revision 9
# speedup vs baseline: 1.0280x; 1.0280x over previous
"""MoE layer (E=8 experts, top-2) on 8 Trainium2 NeuronCores.

Strategy: expert parallelism with host-side routing (the host plays the role
of the all-to-all token dispatch in the sharding hint, exactly like the
host-side combine-sum). Core c holds expert c's weights. The host computes
the router (16 MFLOP), gathers each expert's routed tokens into a fixed
capacity-C buffer, and each core runs the dense FFN over its C tokens:

    yT_c = (gelu(xG_c @ w1_c + b1_c) @ w2_c + b2_c).T     [H, C]

The host scatters the per-core outputs back to token order, scaled by the
top-2 softmax combine weights, and sums the two expert contributions.

Numerics: weights and activations in fp16 (halves weight DMA traffic, which
is otherwise the bottleneck at ~360 GB/s), all matmul accumulation in fp32
PSUM, bias + gelu in fp32 on the scalar engine. Output written in fp32.
"""

import numpy as np

import concourse.mybir as mybir
from concourse import bacc
from concourse.bass_utils import run_bass_kernel_spmd
from concourse.tile import TileContext

FP32 = mybir.dt.float32
FP16 = mybir.dt.float16
AF = mybir.ActivationFunctionType

P = 128
T, H, F, E = 1024, 1024, 4096, 8
HT, FT = H // P, F // P
N_CORES = 8

C_DEFAULT = 272   # expert capacity (max routed load for the fixed input)
NWARM = 5         # PE warmup matmuls to ramp the clock while DMAs land

_cache = {}


def _build_v5(C, act_fn=None):
    act_fn = AF.Gelu if act_fn is None else act_fn
    nc = bacc.Bacc()

    xg = nc.declare_dram_parameter("xg", [P, HT * C], FP16, isOutput=False)
    w1p = nc.declare_dram_parameter("w1p", [P, FT * HT * P], FP16, isOutput=False)
    w2p = nc.declare_dram_parameter("w2p", [P, HT * FT * P], FP16, isOutput=False)
    b1t = nc.declare_dram_parameter("b1t", [P, FT], FP32, isOutput=False)
    b2t = nc.declare_dram_parameter("b2t", [P, HT], FP32, isOutput=False)
    outp = nc.declare_dram_parameter("outp", [H, C], FP32, isOutput=True)

    xg3 = xg.rearrange("p (ht c) -> p ht c", ht=HT)
    w1_4d = w1p.rearrange("p (ft ht fl) -> p ft ht fl", ft=FT, ht=HT)
    w2_4d = w2p.rearrange("p (hh ft hl) -> p hh ft hl", hh=HT, ft=FT)

    # first chunks small so phase-A compute can start early; 2-ft chunks
    # keep DMA supply (~0.73us/ft) ahead of PE demand (~0.91us/ft) with
    # fine availability granularity (each chunk has a +900ns sem lag)
    w1_chunks = [(0, 1), (1, 2), (2, 3), (3, 4)] + [
        (a, min(a + 2, FT)) for a in range(4, FT, 2)
    ]

    with TileContext(nc) as tc:
        with (
            tc.tile_pool(name="const", bufs=1) as const,
            tc.tile_pool(name="wpool", bufs=1) as wpool,
            tc.tile_pool(name="hpool", bufs=1) as hpool,
            tc.tile_pool(name="opool", bufs=3) as opool,
            tc.tile_pool(name="psA", bufs=2, space="PSUM") as psA,
            tc.tile_pool(name="psB", bufs=2, space="PSUM") as psB,
            tc.tile_pool(name="psW", bufs=1, space="PSUM") as psW,
        ):
            # PE warmup: dummy matmuls keep the tensor engine busy (and its
            # p-state ramping) while the first weight/activation DMAs land.
            wmv = const.tile([P, 512], FP16)
            nc.vector.memset(wmv, 0.0)
            pw = psW.tile([P, 512], FP32)
            for _ in range(NWARM):
                nc.tensor.matmul(pw, wmv[:, :P], wmv, start=True, stop=True)

            # DMA issue order == transfer order. Biases first (147ns total,
            # needed by the first activation), then first w1 tile + x so
            # phase-A compute starts as early as possible.
            b1_sb = const.tile([P, FT], FP32)
            nc.sync.dma_start(out=b1_sb, in_=b1t[:, :])
            b2_sb = const.tile([P, HT], FP32)
            nc.sync.dma_start(out=b2_sb, in_=b2t[:, :])
            w1sb = wpool.tile([P, FT, HT, P], FP16)
            a, b = w1_chunks[0]
            nc.sync.dma_start(out=w1sb[:, a:b, :, :], in_=w1_4d[:, a:b, :, :])
            xsb = const.tile([P, HT, C], FP16)
            nc.sync.dma_start(out=xsb[:, : HT // 2, :], in_=xg3[:, : HT // 2, :])
            nc.sync.dma_start(out=xsb[:, HT // 2 :, :], in_=xg3[:, HT // 2 :, :])
            for a, b in w1_chunks[1:]:
                nc.sync.dma_start(out=w1sb[:, a:b, :, :], in_=w1_4d[:, a:b, :, :])
            w2sb = wpool.tile([P, HT, FT, P], FP16)
            for hh in range(HT):
                nc.sync.dma_start(out=w2sb[:, hh, :, :], in_=w2_4d[:, hh, :, :])

            hG = hpool.tile([P, FT, C], FP16)

            # phase A: hG[f, c] = gelu(w1.T @ xG.T + b1), fp16 out
            for ft in range(FT):
                pa = psA.tile([P, C], FP32, tag="pa", name="pa")
                for ht in range(HT):
                    nc.tensor.matmul(
                        pa,
                        w1sb[:, ft, ht, :],
                        xsb[:, ht, :],
                        start=(ht == 0),
                        stop=(ht == HT - 1),
                    )
                nc.scalar.activation(
                    hG[:, ft, :], pa, act_fn, bias=b1_sb[:, ft : ft + 1]
                )

            # phase B: yT[h', c] = w2.T @ hG + b2, fp32 out to DRAM
            for hh in range(HT):
                pb = psB.tile([P, C], FP32, tag="pb", name="pb")
                for ft in range(FT):
                    nc.tensor.matmul(
                        pb,
                        w2sb[:, hh, ft, :],
                        hG[:, ft, :],
                        start=(ft == 0),
                        stop=(ft == FT - 1),
                    )
                yt = opool.tile([P, C], FP32, tag="yt", name="yt")
                nc.scalar.activation(
                    yt, pb, AF.Identity, bias=b2_sb[:, hh : hh + 1]
                )
                nc.sync.dma_start(out=outp[P * hh : P * (hh + 1), :], in_=yt)

    nc.compile()
    return nc


def _get_nc(C=C_DEFAULT):
    key = ("v5", C)
    if key not in _cache:
        _cache[key] = _build_v5(C)
    return _cache[key]


def _route(x_flat, gate_w, gate_b):
    """Top-2 routing on host. Returns per-expert (token idx, combine wt)."""
    logits = x_flat @ gate_w.T + gate_b  # (T, E) fp32
    sel = np.argsort(-logits, axis=1, kind="stable")[:, :2]  # (T, 2)
    tw = np.take_along_axis(logits, sel, axis=1)
    tw = tw - tw.max(axis=1, keepdims=True)
    ew = np.exp(tw)
    rw = ew / ew.sum(axis=1, keepdims=True)  # (T, 2)
    idxs, wts = [], []
    for e in range(E):
        m = sel == e  # (T, 2)
        tok = np.nonzero(m.any(axis=1))[0]
        wt = rw[m.any(axis=1), :][m[m.any(axis=1), :]]
        idxs.append(tok)
        wts.append(wt.astype(np.float32))
    return idxs, wts


def kernel(x, gate_w, gate_b, w1, b1, w2, b2):
    x = np.asarray(x, dtype=np.float32)
    gate_w = np.asarray(gate_w, dtype=np.float32)
    gate_b = np.asarray(gate_b, dtype=np.float32)
    w1 = np.asarray(w1, dtype=np.float32)
    b1 = np.asarray(b1, dtype=np.float32)
    w2 = np.asarray(w2, dtype=np.float32)
    b2 = np.asarray(b2, dtype=np.float32)

    x_flat = x.reshape(T, H)
    idxs, wts = _route(x_flat, gate_w, gate_b)

    max_load = max(len(i) for i in idxs)
    C = C_DEFAULT if max_load <= C_DEFAULT else (max_load + 31) // 32 * 32
    nc = _get_nc(C)

    maps = []
    for c in range(N_CORES):
        tok = idxs[c]
        xg = np.zeros((C, H), dtype=np.float16)
        xg[: len(tok)] = x_flat[tok]
        # [C, H] -> [p, ht, c]
        xgp = np.ascontiguousarray(
            xg.reshape(C, HT, P).transpose(2, 1, 0)
        ).reshape(P, HT * C)
        w1c = w1[c].astype(np.float16)  # [H, F]
        w1pk = np.ascontiguousarray(
            w1c.reshape(HT, P, FT, P).transpose(1, 2, 0, 3)
        ).reshape(P, FT * HT * P)
        w2c = w2[c].astype(np.float16)  # [F, H]
        w2pk = np.ascontiguousarray(
            w2c.reshape(FT, P, HT, P).transpose(1, 2, 0, 3)
        ).reshape(P, HT * FT * P)
        maps.append(
            {
                "xg": xgp,
                "w1p": w1pk,
                "w2p": w2pk,
                "b1t": np.ascontiguousarray(b1[c].reshape(FT, P).T),
                "b2t": np.ascontiguousarray(b2[c].reshape(HT, P).T),
            }
        )

    res = run_bass_kernel_spmd(nc, maps, list(range(N_CORES)))

    out = np.zeros((T, H), dtype=np.float64)
    for c in range(N_CORES):
        yT = res.results[c]["outp"]  # [H, C] fp32
        n = len(idxs[c])
        out[idxs[c]] += wts[c][:, None].astype(np.float64) * yT[:, :n].T
    return out.astype(np.float32).reshape(1, T, H)


# revision 16
# speedup vs baseline: 1.0463x; 1.0178x over previous
"""MoE layer (E=8 experts, top-2) on 8 Trainium2 NeuronCores.

Strategy: expert parallelism with host-side routing (the host plays the role
of the all-to-all token dispatch in the sharding hint, exactly like the
host-side combine-sum). Core c holds expert c's weights. The host computes
the router (16 MFLOP), gathers each expert's routed tokens into a fixed
capacity-C buffer, and each core runs the dense FFN over its C tokens:

    yT_c = (gelu(xG_c @ w1_c + b1_c) @ w2_c + b2_c).T     [H, C]

The host scatters the per-core outputs back to token order, scaled by the
top-2 softmax combine weights, and sums the two expert contributions.

Numerics: weights and activations in fp16 (halves weight DMA traffic, which
is otherwise the bottleneck at ~360 GB/s), all matmul accumulation in fp32
PSUM, bias + gelu in fp32 on the scalar engine. Output written in fp32.
"""

import numpy as np

import concourse.mybir as mybir
from concourse import bacc
from concourse.bass_utils import run_bass_kernel_spmd
from concourse.tile import TileContext

FP32 = mybir.dt.float32
FP16 = mybir.dt.float16
AF = mybir.ActivationFunctionType

P = 128
T, H, F, E = 1024, 1024, 4096, 8
HT, FT = H // P, F // P
N_CORES = 8

C_DEFAULT = 272   # expert capacity (max routed load for the fixed input)
NWARM = 6         # PE warmup matmuls to ramp the clock while DMAs land

_cache = {}


def _build_v5(C, act_fn=None):
    act_fn = AF.Gelu if act_fn is None else act_fn
    nc = bacc.Bacc()

    xg = nc.declare_dram_parameter("xg", [P, HT * C], FP16, isOutput=False)
    w1p = nc.declare_dram_parameter("w1p", [P, FT * HT * P], FP16, isOutput=False)
    w2p = nc.declare_dram_parameter("w2p", [P, HT * FT * P], FP16, isOutput=False)
    bias = nc.declare_dram_parameter("bias", [P, FT + HT], FP32, isOutput=False)
    outp = nc.declare_dram_parameter("outp", [H, C], FP32, isOutput=True)

    xg3 = xg.rearrange("p (ht c) -> p ht c", ht=HT)
    w1_4d = w1p.rearrange("p (ft ht fl) -> p ft ht fl", ft=FT, ht=HT)
    w2_4d = w2p.rearrange("p (hh ft hl) -> p hh ft hl", hh=HT, ft=FT)

    # first chunks small so phase-A compute can start early; 2-ft chunks
    # keep DMA supply (~0.73us/ft) ahead of PE demand (~0.91us/ft) with
    # fine availability granularity (each chunk has a +900ns sem lag)
    w1_chunks = [(0, 1), (1, 2), (2, 3), (3, 4)] + [
        (a, min(a + 2, FT)) for a in range(4, FT, 2)
    ]

    with TileContext(nc) as tc:
        with (
            tc.tile_pool(name="const", bufs=1) as const,
            tc.tile_pool(name="wpool", bufs=1) as wpool,
            tc.tile_pool(name="hpool", bufs=1) as hpool,
            tc.tile_pool(name="opool", bufs=3) as opool,
            tc.tile_pool(name="psA", bufs=2, space="PSUM") as psA,
            tc.tile_pool(name="psB", bufs=2, space="PSUM") as psB,
            tc.tile_pool(name="psW", bufs=1, space="PSUM") as psW,
        ):
            # PE warmup: dummy matmuls keep the tensor engine busy (and its
            # p-state ramping) while the first weight/activation DMAs land.
            wmv = const.tile([P, 512], FP16)
            nc.vector.memset(wmv, 0.0)
            pw = psW.tile([P, 512], FP32)
            for _ in range(NWARM):
                nc.tensor.matmul(pw, wmv[:, :P], wmv, start=True, stop=True)

            # DMA issue order == transfer order: first w1 tile + x first so
            # phase-A compute starts as early as possible; the combined bias
            # row (91ns transfer) slots in before the second w1 chunk --
            # it is needed by the first activation at ~6us.
            w1sb = wpool.tile([P, FT, HT, P], FP16)
            a, b = w1_chunks[0]
            nc.sync.dma_start(out=w1sb[:, a:b, :, :], in_=w1_4d[:, a:b, :, :])
            xsb = const.tile([P, HT, C], FP16)
            nc.sync.dma_start(out=xsb[:, : HT // 2, :], in_=xg3[:, : HT // 2, :])
            nc.sync.dma_start(out=xsb[:, HT // 2 :, :], in_=xg3[:, HT // 2 :, :])
            bias_sb = const.tile([P, FT + HT], FP32)
            nc.sync.dma_start(out=bias_sb, in_=bias[:, :])
            for a, b in w1_chunks[1:]:
                nc.sync.dma_start(out=w1sb[:, a:b, :, :], in_=w1_4d[:, a:b, :, :])
            w2sb = wpool.tile([P, HT, FT, P], FP16)
            for hh in range(HT):
                nc.sync.dma_start(out=w2sb[:, hh, :, :], in_=w2_4d[:, hh, :, :])

            hG = hpool.tile([P, FT, C], FP16)

            # phase A: hG[f, c] = gelu(w1.T @ xG.T + b1), fp16 out
            for ft in range(FT):
                pa = psA.tile([P, C], FP32, tag="pa", name="pa")
                for ht in range(HT):
                    nc.tensor.matmul(
                        pa,
                        w1sb[:, ft, ht, :],
                        xsb[:, ht, :],
                        start=(ht == 0),
                        stop=(ht == HT - 1),
                    )
                nc.scalar.activation(
                    hG[:, ft, :], pa, act_fn, bias=bias_sb[:, ft : ft + 1]
                )

            # phase B: yT[h', c] = w2.T @ hG + b2, fp32 out to DRAM.
            # The final hh is split into two half-C groups so the last
            # output's DMA pipeline overlaps the closing matmuls.
            CH = C // 2
            spans = [(hh, 0, C) for hh in range(HT - 1)]
            spans += [(HT - 1, 0, CH), (HT - 1, CH, C)]
            for hh, c0, c1 in spans:
                pb = psB.tile([P, c1 - c0], FP32, tag="pb", name="pb")
                for ft in range(FT):
                    nc.tensor.matmul(
                        pb,
                        w2sb[:, hh, ft, :],
                        hG[:, ft, c0:c1],
                        start=(ft == 0),
                        stop=(ft == FT - 1),
                    )
                yt = opool.tile([P, c1 - c0], FP32, tag="yt", name="yt")
                nc.scalar.activation(
                    yt, pb, AF.Identity, bias=bias_sb[:, FT + hh : FT + hh + 1]
                )
                nc.sync.dma_start(out=outp[P * hh : P * (hh + 1), c0:c1], in_=yt)

    nc.compile()
    return nc


def _get_nc(C=C_DEFAULT):
    key = ("v5", C)
    if key not in _cache:
        _cache[key] = _build_v5(C)
    return _cache[key]


def _route(x_flat, gate_w, gate_b):
    """Top-2 routing on host. Returns per-expert (token idx, combine wt)."""
    logits = x_flat @ gate_w.T + gate_b  # (T, E) fp32
    sel = np.argsort(-logits, axis=1, kind="stable")[:, :2]  # (T, 2)
    tw = np.take_along_axis(logits, sel, axis=1)
    tw = tw - tw.max(axis=1, keepdims=True)
    ew = np.exp(tw)
    rw = ew / ew.sum(axis=1, keepdims=True)  # (T, 2)
    idxs, wts = [], []
    for e in range(E):
        m = sel == e  # (T, 2)
        tok = np.nonzero(m.any(axis=1))[0]
        wt = rw[m.any(axis=1), :][m[m.any(axis=1), :]]
        idxs.append(tok)
        wts.append(wt.astype(np.float32))
    return idxs, wts


def kernel(x, gate_w, gate_b, w1, b1, w2, b2):
    x = np.asarray(x, dtype=np.float32)
    gate_w = np.asarray(gate_w, dtype=np.float32)
    gate_b = np.asarray(gate_b, dtype=np.float32)
    w1 = np.asarray(w1, dtype=np.float32)
    b1 = np.asarray(b1, dtype=np.float32)
    w2 = np.asarray(w2, dtype=np.float32)
    b2 = np.asarray(b2, dtype=np.float32)

    x_flat = x.reshape(T, H)
    idxs, wts = _route(x_flat, gate_w, gate_b)

    max_load = max(len(i) for i in idxs)
    C = C_DEFAULT if max_load <= C_DEFAULT else (max_load + 31) // 32 * 32
    nc = _get_nc(C)

    maps = []
    for c in range(N_CORES):
        tok = idxs[c]
        xg = np.zeros((C, H), dtype=np.float16)
        xg[: len(tok)] = x_flat[tok]
        # [C, H] -> [p, ht, c]
        xgp = np.ascontiguousarray(
            xg.reshape(C, HT, P).transpose(2, 1, 0)
        ).reshape(P, HT * C)
        w1c = w1[c].astype(np.float16)  # [H, F]
        w1pk = np.ascontiguousarray(
            w1c.reshape(HT, P, FT, P).transpose(1, 2, 0, 3)
        ).reshape(P, FT * HT * P)
        w2c = w2[c].astype(np.float16)  # [F, H]
        w2pk = np.ascontiguousarray(
            w2c.reshape(FT, P, HT, P).transpose(1, 2, 0, 3)
        ).reshape(P, HT * FT * P)
        maps.append(
            {
                "xg": xgp,
                "w1p": w1pk,
                "w2p": w2pk,
                "bias": np.ascontiguousarray(
                    np.concatenate(
                        [b1[c].reshape(FT, P).T, b2[c].reshape(HT, P).T], axis=1
                    )
                ),
            }
        )

    res = run_bass_kernel_spmd(nc, maps, list(range(N_CORES)))

    out = np.zeros((T, H), dtype=np.float64)
    for c in range(N_CORES):
        yT = res.results[c]["outp"]  # [H, C] fp32
        n = len(idxs[c])
        out[idxs[c]] += wts[c][:, None].astype(np.float64) * yT[:, :n].T
    return out.astype(np.float32).reshape(1, T, H)
